# revision 1
# baseline (speedup 1.0000x reference)
"""Trainium2 Bass kernel for nn_Causal_Attention_13082470383895.

Full (unsharded) inputs in, full output out. Internally shards batch*heads
across 8 NeuronCores: core c owns batch c//4 and the 4 heads [4*(c%4), 4*(c%4)+4).
Each core computes its heads' q/k/v projections (column-sharded weights),
QK-layernorm, causal unnormalized-exp attention, and its partial contribution
to the output projection (row-sharded W_out). Host sums the 4 partials per batch.

Hardcoded shapes (per spec): inputs [2, 2048, 1024], W_qk [1024, 2048],
W_v [1024, 1024], W_out [1024, 1024], q/k scale=ones, bias=zeros (per spec
fill; layernorm affine is identity and is not applied).
"""

import os
import sys

import numpy as np

sys.path.insert(0, "/opt/trn_rl_repo")

B = 2
L = 2048
D = 1024
HEADS = 16
DIM = 64
LN_EPS = 1e-6
P = 128
LT = L // P          # 16 l-tiles
DT = D // P          # 8 contraction tiles
NHL = 4              # heads per core
SUP = 4              # 512-wide l_q supertiles
N_CORES = 8

_CACHE = {}


def _make_bacc_cls():
    import bass_rust
    import concourse.mybir as mybir
    from concourse import bacc
    from concourse.hw_specs import get_activation_tables

    class KernelBacc(bacc.Bacc):
        """Bacc whose ACT-table selector never picks the `natural_log` set
        for Ln: hiding `ln` there makes the greedy selector choose
        `natural_log_exp_and_others` (which also holds exp/copy), so the
        kernel needs a single table load instead of thrashing
        exp_and_others <-> natural_log on every layernorm."""

        def insert_act_table_loads(self):
            has_activation = any(
                isinstance(i, mybir.InstActivation)
                for b in self.main_func.blocks
                for i in b.instructions
            )
            if not has_activation:
                return
            ln = mybir.ActivationFunctionType.Ln
            tables = []
            for name, funcs in get_activation_tables(self.m.arch).items():
                if name == "natural_log":
                    funcs = funcs - {ln}
                tables.append((name, funcs))
            bass_rust.insert_act_table_loads(self, tables)

    return KernelBacc


def _build_nc():
    import concourse.bass as bass
    import concourse.mybir as mybir
    import concourse.tile as tile
    from concourse.masks import (
        make_identity,
        make_lower_triangular,
        make_upper_triangular,
    )

    f32 = mybir.dt.float32
    f32r = mybir.dt.float32r
    AF = mybir.ActivationFunctionType
    ALU = mybir.AluOpType

    nc = _make_bacc_cls()("TRN2", target_bir_lowering=False, debug=False)

    X = nc.dram_tensor("x", [L, D], f32, kind="ExternalInput").ap()
    WQK = nc.dram_tensor("w_qk", [D, 512], f32, kind="ExternalInput").ap()
    WV = nc.dram_tensor("w_v", [D, 256], f32, kind="ExternalInput").ap()
    WOUT = nc.dram_tensor("w_out", [256, D], f32, kind="ExternalInput").ap()
    OUT = nc.dram_tensor("out", [L, D], f32, kind="ExternalOutput").ap()

    with tile.TileContext(nc) as tc:
        const = tc.alloc_tile_pool(name="const", bufs=1)
        big = tc.alloc_tile_pool(name="big", bufs=1)
        work = tc.alloc_tile_pool(name="work", bufs=2)
        stat = tc.alloc_tile_pool(name="stat", bufs=3)
        esp = tc.alloc_tile_pool(name="esp", bufs=4)
        outp = tc.alloc_tile_pool(name="outp", bufs=2)

        ident = const.tile([P, P], f32)
        make_identity(nc, ident)
        # S^T layout: element (lk, lq) valid iff lq >= lk. Additive mask
        # applied to scores BEFORE exp: 0 where valid, -1e30 below diagonal.
        maskn = const.tile([P, P], f32)
        make_lower_triangular(nc, maskn, val=-1e30, diag=False)
        ones_f32 = const.tile([P, 1], f32)
        nc.vector.memset(ones_f32, 1.0)
        ones_row = const.tile([1, DIM], f32r)
        nc.vector.tensor_copy(ones_row, ones_f32[0:1, :].to_broadcast([1, DIM]))
        epsb = const.tile([P, 1], f32)
        nc.vector.memset(epsb, float(D * LN_EPS))
        # keep the upper-triangular 0/1 mask for post-exp causal masking
        up01 = const.tile([P, P], f32)
        make_upper_triangular(nc, up01, val=1.0, diag=True)

        # Collapse const-setup waits behind one barrier (wait-slot limits).
        tc.strict_bb_all_engine_barrier()

        # First x tiles before the (bigger) weight DMAs so PE can start
        # transposing immediately.
        x_tiles = {}
        for t in range(2):
            x_t = work.tile([P, D], f32, tag="x", bufs=3, name="x_t")
            nc.sync.dma_start(x_t, X[t * P:(t + 1) * P, :])
            x_tiles[t] = x_t

        # weights: DMA f32, then cast to fp32r (PE operands must be produced
        # as rounded fp32r). wqk cast is chunked so the first projection can
        # start as soon as chunk 0 is ready.
        wqk_f = work.tile([P, DT, 512], f32, tag="wstage", bufs=1)
        nc.sync.dma_start(wqk_f, WQK.rearrange("(o p) n -> p o n", p=P))
        wqk = big.tile([P, DT, 512], f32r)
        for d in range(DT):
            nc.vector.tensor_copy(wqk[:, d], wqk_f[:, d])
        wv_f = work.tile([P, DT, 256], f32, tag="wstage", bufs=1)
        nc.sync.dma_start(wv_f, WV.rearrange("(o p) n -> p o n", p=P))
        wv = big.tile([P, DT, 256], f32r)
        nc.vector.tensor_copy(wv, wv_f)
        wout_f = work.tile([P, 2, D], f32, tag="wstage", bufs=1)
        nc.sync.dma_start(wout_f, WOUT.rearrange("(c p) n -> p c n", p=P))
        wout = big.tile([P, 2, D], f32r)
        nc.vector.tensor_copy(wout, wout_f)

        # persistent intermediates. qt/kt/at pair 2 heads on the partition
        # axis: head 2i in rows 0:64, head 2i+1 in rows 64:128.
        # v is stored augmented per head: [v_h | 1] (65 cols) so one AV
        # matmul yields both the numerator (rows 0:64) and the softmax
        # denominator (row 64).
        v_sb = big.tile([P, LT, NHL, DIM + 1], f32r)
        qt = [big.tile([P, L], f32r, name=f"qt{i}") for i in range(2)]
        kt = [big.tile([P, L], f32r, name=f"kt{i}") for i in range(2)]
        at = [big.tile([P, L], f32r, name=f"at{i}") for i in range(2)]
        # ones column of every v_aug tile (produced as rounded f32r via DVE)
        nc.vector.tensor_copy(
            v_sb[:, :, :, DIM],
            ones_f32[:, 0:1].to_broadcast([P, LT, NHL]),
        )

        # One shared PSUM pool: tags sized so phases A and B can overlap.
        # b512 slots serve xt/proj/qkt/st/av/bc tiles; op gets its own 2
        # banks. 6 + 2 = 8 banks.
        with tc.tile_pool(name="ps", bufs=6, space="PSUM") as ps:
            # Per-supertile: phase A (projections+LN+transposes) for s, then
            # phase B (attention) and C (out-projection) for s — interleaved
            # in program order so the per-engine FIFO streams overlap.
            def phase_a(s):
                qk_tiles = []
                for t in range(4 * s, 4 * s + 4):
                    if t in x_tiles:
                        x_t = x_tiles.pop(t)
                    else:
                        x_t = work.tile([P, D], f32, tag="x", bufs=3,
                                        name="x_t")
                        nc.sync.dma_start(x_t, X[t * P:(t + 1) * P, :])

                    # transpose x tile -> x^T chunks [d, l]
                    xt_sb = work.tile([P, DT, P], f32r, tag="xt_sb")
                    for half in range(2):
                        xt_ps = ps.tile([P, 512], f32, tag="b512",
                                        name="xt_ps")
                        for dj in range(4):
                            d = half * 4 + dj
                            nc.tensor.transpose(
                                xt_ps[:, dj * P:(dj + 1) * P],
                                x_t[:, d * P:(d + 1) * P],
                                ident,
                            )
                        if half == 0:
                            nc.scalar.copy(
                                xt_sb[:, :4, :],
                                xt_ps.rearrange("p (a b) -> p a b", a=4),
                            )
                        else:
                            nc.vector.tensor_copy(
                                xt_sb[:, 4:, :],
                                xt_ps.rearrange("p (a b) -> p a b", a=4),
                            )

                    # qk / v projections (contract over D)
                    qk_ps = ps.tile([P, 512], f32, tag="b512", name="qk_ps")
                    v_ps = ps.tile([P, 512], f32, tag="b512", name="v_ps")
                    for d in range(DT):
                        nc.tensor.matmul(
                            qk_ps, xt_sb[:, d], wqk[:, d],
                            start=(d == 0), stop=(d == DT - 1),
                        )
                    for d in range(DT):
                        nc.tensor.matmul(
                            v_ps[:, :256], xt_sb[:, d], wv[:, d],
                            start=(d == 0), stop=(d == DT - 1),
                        )
                    # 72-wide groups: pad so per-group APs stay 3D
                    qk_full = work.tile([P, 8, DIM + 8], f32, tag="qk_sb",
                                        bufs=6)
                    qk_sb = qk_full[:, :, :DIM]
                    nc.vector.tensor_copy(
                        qk_sb, qk_ps.rearrange("p (g d) -> p g d", g=8))
                    nc.vector.tensor_copy(
                        v_sb[:, t, :, :DIM],
                        v_ps[:, :256].rearrange("p (h d) -> p h d", h=NHL))

                    # layernorm over each 64-group. qk is RAW (unscaled by
                    # 1/32): (raw-m)/sqrt(var_raw + 1024*eps) matches the
                    # reference exactly.
                    bnst_full = stat.tile([P, 8, 8], f32, tag="bnst")
                    bnst = bnst_full[:, :, :6]
                    mv = stat.tile([P, 8, 2], f32, tag="mv")
                    for g in range(8):
                        nc.vector.bn_stats(bnst[:, g], qk_sb[:, g])
                        nc.vector.bn_aggr(mv[:, g], bnst[:, g])
                    rstd = stat.tile([P, 8], f32, tag="rstd")
                    nc.scalar.activation(rstd, mv[:, :, 1], AF.Ln,
                                         bias=epsb, scale=1.0)
                    nc.scalar.activation(rstd, rstd, AF.Exp, scale=-0.5)
                    prod = stat.tile([P, 8], f32, tag="prod")
                    nc.vector.tensor_tensor(prod, mv[:, :, 0], rstd, ALU.mult)
                    for g in range(8):
                        nc.gpsimd.tensor_scalar(
                            qk_sb[:, g], qk_sb[:, g],
                            rstd[:, g:g + 1], prod[:, g:g + 1],
                            op0=ALU.mult, op1=ALU.subtract,
                        )
                    qk_tiles.append(qk_sb)

                # transpose q_n, k_n -> [dim, l] for this supertile's 4
                # l-tiles. Matmul outputs must start at PSUM partition 0, so
                # transpose into [64, 512] tiles and pair heads during the
                # SBUF copy.
                for hl in range(NHL):
                    pr, ro = hl // 2, DIM * (hl % 2)
                    for which, dst in ((0, qt), (1, kt)):
                        tp_ps = ps.tile([DIM, 512], f32, tag="b512",
                                        name="tp_ps")
                        for i in range(4):
                            nc.tensor.transpose(
                                tp_ps[:, i * P:(i + 1) * P],
                                qk_tiles[i][:, 2 * hl + which],
                                ident,
                            )
                        nc.vector.tensor_copy(
                            dst[pr][ro:ro + DIM, s * 512:(s + 1) * 512],
                            tp_ps,
                        )

            def phase_bc(s):
                ls = slice(s * 512, (s + 1) * 512)
                njs = 4 * s + 4
                for pr in range(2):
                    # two heads interleaved: disjoint PE row groups (0:64 /
                    # 64:128) let their K=64 QK matmuls run concurrently
                    av_list = []
                    for r01 in range(2):
                        av_list.append(ps.tile([DIM + 1, 512], f32,
                                               tag="b512",
                                               name=f"av_ps{r01}"))
                    for j in range(njs):
                        pp = j - 4 * s  # >=0: diagonal tile needing mask
                        woff = max(0, pp) * P
                        es_list = []
                        for r01 in range(2):
                            ro = DIM * r01
                            st_ps = ps.tile([P, 512], f32, tag="b512",
                                            name=f"st_ps{r01}")
                            nc.tensor.matmul(
                                st_ps,
                                kt[pr][ro:ro + DIM, j * P:(j + 1) * P],
                                qt[pr][ro:ro + DIM, ls],
                                start=True, stop=True, tile_position=(ro, 0),
                            )
                            es = esp.tile([P, 512], f32r, tag="es")
                            nc.scalar.activation(es[:, woff:],
                                                 st_ps[:, woff:],
                                                 AF.Exp, scale=1.0 / DIM)
                            if pp >= 0:
                                blk = slice(pp * P, (pp + 1) * P)
                                nc.gpsimd.tensor_tensor(
                                    es[:, blk], es[:, blk], up01, ALU.mult)
                            es_list.append(es)
                        for r01 in range(2):
                            hl = 2 * pr + r01
                            nc.tensor.matmul(
                                av_list[r01][:, woff:],
                                v_sb[:, j, hl],
                                es_list[r01][:, woff:],
                                start=(j == 0), stop=(j == njs - 1),
                            )
                    for r01 in range(2):
                        hl = 2 * pr + r01
                        ro = DIM * r01
                        av_ps = av_list[r01]
                        recip = stat.tile([1, 512], f32r, tag="recip")
                        with nc.allow_low_precision(
                                reason="fp32r rounding of softmax recip"):
                            nc.vector.reciprocal(recip, av_ps[DIM:DIM + 1, :])
                        bc_ps = ps.tile([DIM, 512], f32, tag="b512",
                                        name="bc_ps")
                        nc.tensor.matmul(bc_ps, ones_row, recip,
                                         start=True, stop=True)
                        # DVE reads at most one PSUM operand: stage av via
                        # ScalarE
                        av_sb = esp.tile([DIM, 512], f32, tag="avsb")
                        nc.scalar.copy(av_sb, av_ps[:DIM])
                        nc.vector.tensor_tensor(at[pr][ro:ro + DIM, ls],
                                                av_sb, bc_ps, ALU.mult)

            def phase_c(s):
                # output projection for supertile s's l-tiles
                for t in range(4 * s, 4 * s + 4):
                    op_ps = ps.tile([P, D], f32, tag="op", bufs=1,
                                    name="op_ps")
                    for nch in range(2):
                        for c in range(2):
                            nc.tensor.matmul(
                                op_ps[:, nch * 512:(nch + 1) * 512],
                                at[c][:, t * P:(t + 1) * P],
                                wout[:, c, nch * 512:(nch + 1) * 512],
                                start=(c == 0), stop=(c == 1),
                            )
                    o_sb = outp.tile([P, D], f32, tag="o")
                    # 1/32 (v proj) * 1/32 (out proj) = 1/1024
                    nc.scalar.mul(o_sb, op_ps, 1.0 / 1024.0)
                    nc.sync.dma_start(OUT[t * P:(t + 1) * P, :], o_sb)

            for s in range(SUP):
                phase_a(s)
                phase_bc(s)
                if s > 0:
                    phase_c(s - 1)
            phase_c(SUP - 1)

        outp.release()
        esp.release()
        stat.release()
        work.release()
        big.release()
        const.release()

    nc.finalize()
    return nc


def _get_nc():
    if "nc" not in _CACHE:
        _CACHE["nc"] = _build_nc()
    return _CACHE["nc"]


def kernel(**inputs):
    x = np.ascontiguousarray(np.asarray(inputs["inputs"], dtype=np.float32))
    w_qk = np.asarray(inputs["W_qk"], dtype=np.float32)
    w_v = np.asarray(inputs["W_v"], dtype=np.float32)
    w_out = np.asarray(inputs["W_out"], dtype=np.float32)

    nc = _get_nc()
    in_maps = []
    for c in range(N_CORES):
        b, g = divmod(c, 4)
        in_maps.append({
            "x": np.ascontiguousarray(x[b]),
            "w_qk": np.ascontiguousarray(w_qk[:, 512 * g:512 * (g + 1)]),
            "w_v": np.ascontiguousarray(w_v[:, 256 * g:256 * (g + 1)]),
            "w_out": np.ascontiguousarray(w_out[256 * g:256 * (g + 1), :]),
        })

    from concourse.bass_utils import run_bass_kernel_spmd

    trace = bool(os.environ.get("KERNEL_TRACE"))
    if trace:
        try:
            from antenv.axon_hooks import get_axon_ntff_profile_hook  # noqa: F401
        except Exception:
            trace = False
    res = run_bass_kernel_spmd(nc, in_maps, core_ids=list(range(N_CORES)),
                               trace=trace)
    _CACHE["last_results"] = res
    outs = [m["out"] for m in res.results]
    out = np.stack([
        outs[0] + outs[1] + outs[2] + outs[3],
        outs[4] + outs[5] + outs[6] + outs[7],
    ]).astype(np.float32)
    return out



# revision 3
# speedup vs baseline: 1.6545x; 1.6545x over previous
"""Trainium2 Bass kernel for nn_Causal_Attention_13082470383895.

Full (unsharded) inputs in, full output out. Internally shards batch*heads
across 8 NeuronCores: core c owns batch c//4 and the 4 heads [4*(c%4), 4*(c%4)+4).
Each core computes its heads' q/k/v projections (column-sharded weights),
QK-layernorm, causal unnormalized-exp attention, and its partial contribution
to the output projection (row-sharded W_out). Host sums the 4 partials per batch.

All matmul operands are bf16 (PSUM accumulates fp32). x is pre-transposed and
cast to bf16 on the host, so the kernel needs no x transposes. Output partials
are bf16; the host sums them in fp32.

Hardcoded shapes (per spec): inputs [2, 2048, 1024], W_qk [1024, 2048],
W_v [1024, 1024], W_out [1024, 1024], q/k scale=ones, bias=zeros (per spec
fill; layernorm affine is identity and is not applied).
"""

import os
import sys

import numpy as np

sys.path.insert(0, "/opt/trn_rl_repo")

B = 2
L = 2048
D = 1024
HEADS = 16
DIM = 64
LN_EPS = 1e-6
P = 128
LT = L // P          # 16 l-tiles
DT = D // P          # 8 contraction tiles
NHL = 4              # heads per core
SUP = 4              # 512-wide l_q supertiles
N_CORES = 8

_CACHE = {}


def _make_bacc_cls():
    import bass_rust
    import concourse.mybir as mybir
    from concourse import bacc
    from concourse.hw_specs import get_activation_tables

    class KernelBacc(bacc.Bacc):
        """Bacc whose ACT-table selector never picks the `natural_log` set
        for Ln: hiding `ln` there makes the greedy selector choose
        `natural_log_exp_and_others` (which also holds exp/copy), so the
        kernel needs a single table load instead of thrashing
        exp_and_others <-> natural_log on every layernorm."""

        def insert_act_table_loads(self):
            has_activation = any(
                isinstance(i, mybir.InstActivation)
                for b in self.main_func.blocks
                for i in b.instructions
            )
            if not has_activation:
                return
            ln = mybir.ActivationFunctionType.Ln
            tables = []
            for name, funcs in get_activation_tables(self.m.arch).items():
                if name == "natural_log":
                    funcs = funcs - {ln}
                tables.append((name, funcs))
            bass_rust.insert_act_table_loads(self, tables)

    return KernelBacc


def _build_nc():
    import concourse.bass as bass
    import concourse.mybir as mybir
    import concourse.tile as tile
    from concourse.masks import make_identity, make_upper_triangular

    f32 = mybir.dt.float32
    bf16 = mybir.dt.bfloat16
    AF = mybir.ActivationFunctionType
    ALU = mybir.AluOpType

    nc = _make_bacc_cls()("TRN2", target_bir_lowering=False, debug=False)

    XT = nc.dram_tensor("xt", [D, L], bf16, kind="ExternalInput").ap()
    WQK = nc.dram_tensor("w_qk", [D, 512], bf16, kind="ExternalInput").ap()
    WV = nc.dram_tensor("w_v", [D, 256], bf16, kind="ExternalInput").ap()
    WOUT = nc.dram_tensor("w_out", [256, D], bf16, kind="ExternalInput").ap()
    OUT = nc.dram_tensor("out", [L, D], bf16, kind="ExternalOutput").ap()

    with tile.TileContext(nc) as tc:
        const = tc.alloc_tile_pool(name="const", bufs=1)
        big = tc.alloc_tile_pool(name="big", bufs=1)
        work = tc.alloc_tile_pool(name="work", bufs=2)
        stat = tc.alloc_tile_pool(name="stat", bufs=3)
        esp = tc.alloc_tile_pool(name="esp", bufs=4)
        outp = tc.alloc_tile_pool(name="outp", bufs=2)

        ident = const.tile([P, P], bf16)
        make_identity(nc, ident)
        # upper-triangular (incl diag) 0/1 mask for post-exp causal masking of
        # the diagonal 128x128 block (S^T layout: valid iff lq >= lk).
        up01 = const.tile([P, P], bf16)
        make_upper_triangular(nc, up01, val=1.0, diag=True)
        ones_row = const.tile([1, DIM], bf16)
        nc.vector.memset(ones_row, 1.0)
        epsb = const.tile([P, 1], f32)
        nc.vector.memset(epsb, float(D * LN_EPS))

        # Collapse const-setup waits behind one barrier (wait-slot limits).
        tc.strict_bb_all_engine_barrier()

        # x^T [d, l] staged fully in SBUF, DMA'd per 128-row chunk so the
        # first projections start as soon as chunk 0 lands.
        xt = big.tile([P, DT, L], bf16)
        xt_src = XT.rearrange("(c p) l -> p c l", p=P)
        for c in range(DT):
            nc.sync.dma_start(xt[:, c], xt_src[:, c])
        wqk = big.tile([P, DT, 512], bf16)
        nc.sync.dma_start(wqk, WQK.rearrange("(c p) n -> p c n", p=P))
        wv = big.tile([P, DT, 256], bf16)
        nc.sync.dma_start(wv, WV.rearrange("(c p) n -> p c n", p=P))
        wout = big.tile([P, 2, D], bf16)
        nc.sync.dma_start(wout, WOUT.rearrange("(c p) n -> p c n", p=P))

        # persistent intermediates. qt/kt/at pair 2 heads on the partition
        # axis: head 2i in rows 0:64, head 2i+1 in rows 64:128.
        # v is stored augmented per head: [v_h | 1] (65 cols) so one AV
        # matmul yields both the numerator (rows 0:64) and the softmax
        # denominator (row 64).
        v_sb = big.tile([P, LT, NHL, DIM + 1], bf16)
        qt = [big.tile([P, L], bf16, name=f"qt{i}") for i in range(2)]
        kt = [big.tile([P, L], bf16, name=f"kt{i}") for i in range(2)]
        at = [big.tile([P, L], bf16, name=f"at{i}") for i in range(2)]
        nc.vector.memset(v_sb[:, :, :, DIM], 1.0)

        # One shared PSUM pool: b512 slots serve proj/transpose/qk/av/bc
        # tiles; op gets its own 2 banks. 6 + 2 = 8 banks.
        with tc.tile_pool(name="ps", bufs=6, space="PSUM") as ps:

            def phase_a(s):
                qn_tiles = []
                for t in range(4 * s, 4 * s + 4):
                    tsl = slice(t * P, (t + 1) * P)
                    qk_ps = ps.tile([P, 512], f32, tag="b512", name="qk_ps")
                    for c in range(DT):
                        nc.tensor.matmul(
                            qk_ps, xt[:, c, tsl], wqk[:, c],
                            start=(c == 0), stop=(c == DT - 1),
                        )
                    v_ps = ps.tile([P, 512], f32, tag="b512", name="v_ps")
                    for c in range(DT):
                        nc.tensor.matmul(
                            v_ps[:, :256], xt[:, c, tsl], wv[:, c],
                            start=(c == 0), stop=(c == DT - 1),
                        )

                    # layernorm over each 64-group. qk is RAW (unscaled by
                    # 1/32): (raw-m)/sqrt(var_raw + 1024*eps) matches the
                    # reference exactly.
                    qk3 = qk_ps.rearrange("p (g d) -> p g d", g=8)
                    bnst = stat.tile([P, 8, 6], f32, tag="bnst")
                    mv = stat.tile([P, 8, 2], f32, tag="mv")
                    for g in range(8):
                        nc.vector.bn_stats(bnst[:, g], qk3[:, g])
                        nc.vector.bn_aggr(mv[:, g], bnst[:, g])
                    rstd = stat.tile([P, 8], f32, tag="rstd")
                    nc.scalar.activation(rstd, mv[:, :, 1], AF.Ln,
                                         bias=epsb, scale=1.0)
                    nc.scalar.activation(rstd, rstd, AF.Exp, scale=-0.5)
                    prod = stat.tile([P, 8], f32, tag="prod")
                    nc.vector.tensor_tensor(prod, mv[:, :, 0], rstd, ALU.mult)
                    qn = work.tile([P, 8, DIM], bf16, tag="qn", bufs=6,
                                   name="qn")
                    for g in range(8):
                        nc.vector.tensor_scalar(
                            qn[:, g], qk3[:, g],
                            rstd[:, g:g + 1], prod[:, g:g + 1],
                            op0=ALU.mult, op1=ALU.subtract,
                        )
                    nc.vector.tensor_copy(
                        v_sb[:, t, :, :DIM],
                        v_ps[:, :256].rearrange("p (h d) -> p h d", h=NHL))
                    qn_tiles.append(qn)

                # transpose q_n, k_n -> [dim, l] for this supertile's 4
                # l-tiles. Matmul outputs must start at PSUM partition 0, so
                # transpose into [64, 512] tiles and pair heads during the
                # SBUF copy.
                for hl in range(NHL):
                    pr, ro = hl // 2, DIM * (hl % 2)
                    for which, dst in ((0, qt), (1, kt)):
                        tp_ps = ps.tile([DIM, 512], bf16, tag="b512",
                                        name="tp_ps")
                        for i in range(4):
                            nc.tensor.transpose(
                                tp_ps[:, i * P:(i + 1) * P],
                                qn_tiles[i][:, 2 * hl + which],
                                ident,
                            )
                        nc.vector.tensor_copy(
                            dst[pr][ro:ro + DIM, s * 512:(s + 1) * 512],
                            tp_ps,
                        )

            def phase_b(s):
                ls = slice(s * 512, (s + 1) * 512)
                njs = 4 * s + 4
                for pr in range(2):
                    # two heads interleaved on disjoint PE row groups
                    # (0:64 / 64:128). The j-loop is software-pipelined:
                    # QK for tile j+1 is issued before AV for tile j so the
                    # PE never stalls waiting on the ACT exp.
                    av_list = [
                        ps.tile([DIM + 1, 512], f32, tag="b512",
                                name=f"av_ps{r01}")
                        for r01 in range(2)
                    ]

                    def issue_qk_exp(j):
                        pp = j - 4 * s  # >=0: diagonal tile needing mask
                        woff = max(0, pp) * P
                        es_list = []
                        for r01 in range(2):
                            ro = DIM * r01
                            st_ps = ps.tile([P, 512], f32, tag="b512",
                                            name=f"st_ps{r01}")
                            nc.tensor.matmul(
                                st_ps,
                                kt[pr][ro:ro + DIM, j * P:(j + 1) * P],
                                qt[pr][ro:ro + DIM, ls],
                                start=True, stop=True, tile_position=(ro, 0),
                            )
                            es = esp.tile([P, 512], bf16, tag="es")
                            nc.scalar.activation(es[:, woff:],
                                                 st_ps[:, woff:],
                                                 AF.Exp, scale=1.0 / DIM)
                            if pp >= 0:
                                blk = slice(pp * P, (pp + 1) * P)
                                nc.vector.tensor_tensor(
                                    es[:, blk], es[:, blk], up01, ALU.mult)
                            es_list.append(es)
                        return woff, es_list

                    def issue_av(j, woff, es_list):
                        for r01 in range(2):
                            hl = 2 * pr + r01
                            nc.tensor.matmul(
                                av_list[r01][:, woff:],
                                v_sb[:, j, hl],
                                es_list[r01][:, woff:],
                                start=(j == 0), stop=(j == njs - 1),
                            )

                    pend = issue_qk_exp(0)
                    for j in range(1, njs):
                        nxt = issue_qk_exp(j)
                        issue_av(j - 1, *pend)
                        pend = nxt
                    issue_av(njs - 1, *pend)

                    for r01 in range(2):
                        ro = DIM * r01
                        av_ps = av_list[r01]
                        recip = stat.tile([1, 512], bf16, tag="recip")
                        with nc.allow_low_precision(
                                reason="bf16 rounding of softmax recip"):
                            nc.vector.reciprocal(recip, av_ps[DIM:DIM + 1, :])
                        bc_ps = ps.tile([DIM, 512], f32, tag="b512",
                                        name="bc_ps")
                        nc.tensor.matmul(bc_ps, ones_row, recip,
                                         start=True, stop=True)
                        av_sb = esp.tile([DIM, 512], bf16, tag="avsb")
                        nc.vector.tensor_copy(av_sb, av_ps[:DIM])
                        nc.vector.tensor_tensor(at[pr][ro:ro + DIM, ls],
                                                av_sb, bc_ps, ALU.mult)

            def phase_c(s):
                # output projection for supertile s's l-tiles
                for t in range(4 * s, 4 * s + 4):
                    op_ps = ps.tile([P, D], f32, tag="op", bufs=1,
                                    name="op_ps")
                    for nch in range(2):
                        for c in range(2):
                            nc.tensor.matmul(
                                op_ps[:, nch * 512:(nch + 1) * 512],
                                at[c][:, t * P:(t + 1) * P],
                                wout[:, c, nch * 512:(nch + 1) * 512],
                                start=(c == 0), stop=(c == 1),
                            )
                    o_sb = outp.tile([P, D], bf16, tag="o")
                    # 1/32 (v proj) * 1/32 (out proj) = 1/1024
                    nc.scalar.mul(o_sb, op_ps, 1.0 / 1024.0)
                    nc.sync.dma_start(OUT[t * P:(t + 1) * P, :], o_sb)

            for s in range(SUP):
                phase_a(s)
                phase_b(s)
                if s > 0:
                    phase_c(s - 1)
            phase_c(SUP - 1)

        outp.release()
        esp.release()
        stat.release()
        work.release()
        big.release()
        const.release()

    nc.finalize()
    return nc


def _get_nc():
    if "nc" not in _CACHE:
        _CACHE["nc"] = _build_nc()
    return _CACHE["nc"]


def kernel(**inputs):
    import ml_dtypes

    bf16 = ml_dtypes.bfloat16
    x = np.asarray(inputs["inputs"], dtype=np.float32)
    w_qk = np.asarray(inputs["W_qk"], dtype=np.float32)
    w_v = np.asarray(inputs["W_v"], dtype=np.float32)
    w_out = np.asarray(inputs["W_out"], dtype=np.float32)

    xt = [np.ascontiguousarray(x[b].T).astype(bf16) for b in range(B)]

    nc = _get_nc()
    in_maps = []
    for c in range(N_CORES):
        b, g = divmod(c, 4)
        in_maps.append({
            "xt": xt[b],
            "w_qk": np.ascontiguousarray(
                w_qk[:, 512 * g:512 * (g + 1)]).astype(bf16),
            "w_v": np.ascontiguousarray(
                w_v[:, 256 * g:256 * (g + 1)]).astype(bf16),
            "w_out": np.ascontiguousarray(
                w_out[256 * g:256 * (g + 1), :]).astype(bf16),
        })

    from concourse.bass_utils import run_bass_kernel_spmd

    trace = bool(os.environ.get("KERNEL_TRACE"))
    if trace:
        try:
            from antenv.axon_hooks import get_axon_ntff_profile_hook
            if get_axon_ntff_profile_hook() is None:
                trace = False
        except Exception:
            trace = False
    res = run_bass_kernel_spmd(nc, in_maps, core_ids=list(range(N_CORES)),
                               trace=trace)
    _CACHE["last_results"] = res
    outs = [m["out"].astype(np.float32) for m in res.results]
    out = np.stack([
        outs[0] + outs[1] + outs[2] + outs[3],
        outs[4] + outs[5] + outs[6] + outs[7],
    ]).astype(np.float32)
    return out


# revision 18
# speedup vs baseline: 1.6629x; 1.0051x over previous
"""Trainium2 Bass kernel for nn_Causal_Attention_13082470383895.

Full (unsharded) inputs in, full output out. Internally shards batch*heads
across 8 NeuronCores: core c owns batch c//4 and the 4 heads [4*(c%4), 4*(c%4)+4).
Each core computes its heads' q/k/v projections (column-sharded weights),
QK-layernorm, causal unnormalized-exp attention, and its partial contribution
to the output projection (row-sharded W_out). Host sums the 4 partials per batch.

All matmul operands are bf16 (PSUM accumulates fp32). x is pre-transposed and
cast to bf16 on the host, so the kernel needs no x transposes. Output partials
are bf16; the host sums them in fp32.

Pipeline notes: the attention j-loop is software-pipelined two tiles deep
(QK for j+2 issues before AV for j) so the PE never waits on the ACT exp;
the out-projection for supertile s-1 is emitted inside phase_a(s) as PE
filler while the layernorm chain (DVE/ACT) catches up; softmax reciprocal
uses the fast custom-DVE approximation (the exact DVE reciprocal costs
~6.5ns/element on a 1-partition AP).

Hardcoded shapes (per spec): inputs [2, 2048, 1024], W_qk [1024, 2048],
W_v [1024, 1024], W_out [1024, 1024], q/k scale=ones, bias=zeros (per spec
fill; layernorm affine is identity and is not applied).
"""

import math
import os
import sys

import numpy as np

sys.path.insert(0, "/opt/trn_rl_repo")

B = 2
L = 2048
D = 1024
HEADS = 16
DIM = 64
LN_EPS = 1e-6
P = 128
LT = L // P          # 16 l-tiles
DT = D // P          # 8 contraction tiles
NHL = 4              # heads per core
SUP = 4              # 512-wide l_q supertiles
N_CORES = 8

_CACHE = {}


def _make_bacc_cls():
    import bass_rust
    import concourse.mybir as mybir
    from concourse import bacc
    from concourse.hw_specs import get_activation_tables

    class KernelBacc(bacc.Bacc):
        """Bacc whose ACT-table selector never picks the `natural_log` set
        for Ln: hiding `ln` there makes the greedy selector choose
        `natural_log_exp_and_others` (which also holds exp/copy/identity/
        square), so the kernel needs a single table load."""

        def insert_act_table_loads(self):
            has_activation = any(
                isinstance(i, mybir.InstActivation)
                for b in self.main_func.blocks
                for i in b.instructions
            )
            if not has_activation:
                return
            ln = mybir.ActivationFunctionType.Ln
            tables = []
            for name, funcs in get_activation_tables(self.m.arch).items():
                if name == "natural_log":
                    funcs = funcs - {ln}
                tables.append((name, funcs))
            bass_rust.insert_act_table_loads(self, tables)

    return KernelBacc


def _build_nc():
    import concourse.bass as bass
    import concourse.mybir as mybir
    import concourse.tile as tile
    from concourse.masks import make_identity, make_upper_triangular

    f32 = mybir.dt.float32
    bf16 = mybir.dt.bfloat16
    AF = mybir.ActivationFunctionType
    ALU = mybir.AluOpType
    AX = mybir.AxisListType

    # layernorm over raw (unscaled) qk groups of 64:
    # (raw - m) / sqrt(var_raw + 1024*eps)  with  m2 = 64*var_raw
    #   rstd = 8 / sqrt(m2 + 64*1024*eps) = exp(-0.5*ln(m2 + EPS2) + ln 8)
    EPS2 = float(DIM * D * LN_EPS)      # 0.065536
    LN8 = float(math.log(8.0))

    nc = _make_bacc_cls()("TRN2", target_bir_lowering=False, debug=False)

    XT = nc.dram_tensor("xt", [D, L], bf16, kind="ExternalInput").ap()
    WQK = nc.dram_tensor("w_qk", [D, 512], bf16, kind="ExternalInput").ap()
    WV = nc.dram_tensor("w_v", [D, 256], bf16, kind="ExternalInput").ap()
    WOUT = nc.dram_tensor("w_out", [256, D], bf16, kind="ExternalInput").ap()
    OUT = nc.dram_tensor("out", [L, D], bf16, kind="ExternalOutput").ap()

    with tile.TileContext(nc) as tc:
        const = tc.alloc_tile_pool(name="const", bufs=1)
        big = tc.alloc_tile_pool(name="big", bufs=1)
        work = tc.alloc_tile_pool(name="work", bufs=2)
        stat = tc.alloc_tile_pool(name="stat", bufs=3)
        esp = tc.alloc_tile_pool(name="esp", bufs=6)
        outp = tc.alloc_tile_pool(name="outp", bufs=2)

        ident = const.tile([P, P], bf16)
        make_identity(nc, ident)
        up01 = const.tile([P, P], bf16)
        make_upper_triangular(nc, up01, val=1.0, diag=True)
        ones_row = const.tile([1, DIM], f32)
        nc.vector.memset(ones_row, 1.0)
        eps2b = const.tile([P, 1], f32)
        nc.vector.memset(eps2b, EPS2)
        ln8b = const.tile([P, 1], f32)
        nc.vector.memset(ln8b, LN8)

        tc.strict_bb_all_engine_barrier()

        xt = big.tile([P, DT, L], bf16)
        xt_src = XT.rearrange("(c p) l -> p c l", p=P)
        for c in range(DT):
            nc.sync.dma_start(xt[:, c], xt_src[:, c])
        wqk = big.tile([P, DT, 512], bf16)
        nc.sync.dma_start(wqk, WQK.rearrange("(c p) n -> p c n", p=P))
        wv = big.tile([P, DT, 256], bf16)
        nc.sync.dma_start(wv, WV.rearrange("(c p) n -> p c n", p=P))
        wout = big.tile([P, 2, D], bf16)
        nc.sync.dma_start(wout, WOUT.rearrange("(c p) n -> p c n", p=P))

        v_sb = big.tile([P, LT, NHL, DIM + 1], bf16)
        qt = [big.tile([P, L], bf16, name=f"qt{i}") for i in range(2)]
        kt = [big.tile([P, L], bf16, name=f"kt{i}") for i in range(2)]
        at = [big.tile([P, L], bf16, name=f"at{i}") for i in range(2)]
        nc.vector.memset(v_sb[:, :, :, DIM], 1.0)

        with tc.tile_pool(name="ps", bufs=6, space="PSUM") as ps:

            def proj_qk_ln(s):
                """qk projections + layernorm for supertile s's 4 l-tiles.
                Returns the 4 normalized bf16 tiles [P, 8, 64]."""
                qn_tiles = []
                for t in range(4 * s, 4 * s + 4):
                    tsl = slice(t * P, (t + 1) * P)
                    qk_ps = ps.tile([P, 512], f32, tag="b512", name="qk_ps")
                    for c in range(DT):
                        nc.tensor.matmul(
                            qk_ps, xt[:, c, tsl], wqk[:, c],
                            start=(c == 0), stop=(c == DT - 1),
                        )
                    qk3 = qk_ps.rearrange("p (g d) -> p g d", g=8)
                    sum_ = stat.tile([P, 8], f32, tag="sum")
                    nc.vector.tensor_reduce(sum_, qk3, AX.X, ALU.add)
                    sq = work.tile([P, 8, DIM], f32, tag="sq", bufs=2)
                    nc.scalar.activation(sq, qk3, AF.Square)
                    ssq = stat.tile([P, 8], f32, tag="ssq")
                    nc.vector.tensor_reduce(ssq, sq, AX.X, ALU.add)
                    t1 = stat.tile([P, 8], f32, tag="t1")
                    nc.vector.tensor_tensor(t1, sum_, sum_, ALU.mult)
                    m2 = stat.tile([P, 8], f32, tag="m2")
                    nc.vector.scalar_tensor_tensor(
                        m2, t1, -1.0 / DIM, ssq, op0=ALU.mult, op1=ALU.add)
                    rstd = stat.tile([P, 8], f32, tag="rstd")
                    nc.scalar.activation(rstd, m2, AF.Ln, bias=eps2b,
                                         scale=1.0)
                    nc.scalar.activation(rstd, rstd, AF.Exp,
                                         bias=ln8b, scale=-0.5)
                    negprod = stat.tile([P, 8], f32, tag="negprod")
                    nc.vector.scalar_tensor_tensor(
                        negprod, sum_, -1.0 / DIM, rstd,
                        op0=ALU.mult, op1=ALU.mult)
                    qn = work.tile([P, 8, DIM], bf16, tag="qn", bufs=6,
                                   name="qn")
                    for g in range(8):
                        nc.scalar.activation(
                            qn[:, g], qk3[:, g], AF.Identity,
                            bias=negprod[:, g:g + 1], scale=rstd[:, g:g + 1])
                    qn_tiles.append(qn)
                return qn_tiles

            def proj_v(s):
                for t in range(4 * s, 4 * s + 4):
                    tsl = slice(t * P, (t + 1) * P)
                    v_ps = ps.tile([P, 512], f32, tag="b512", name="v_ps")
                    for c in range(DT):
                        nc.tensor.matmul(
                            v_ps[:, :256], xt[:, c, tsl], wv[:, c],
                            start=(c == 0), stop=(c == DT - 1),
                        )
                    nc.vector.tensor_copy(
                        v_sb[:, t, :, :DIM],
                        v_ps[:, :256].rearrange("p (h d) -> p h d", h=NHL))

            def transposes(s, qn_tiles):
                for hl in range(NHL):
                    pr, ro = hl // 2, DIM * (hl % 2)
                    for which, dst in ((0, qt), (1, kt)):
                        tp_ps = ps.tile([DIM, 512], bf16, tag="b512",
                                        name="tp_ps")
                        for i in range(4):
                            nc.tensor.transpose(
                                tp_ps[:, i * P:(i + 1) * P],
                                qn_tiles[i][:, 2 * hl + which],
                                ident,
                            )
                        nc.vector.tensor_copy(
                            dst[pr][ro:ro + DIM, s * 512:(s + 1) * 512],
                            tp_ps,
                        )

            def phase_b(s):
                ls = slice(s * 512, (s + 1) * 512)
                njs = 4 * s + 4
                av_pair = {}
                dbg = os.environ.get("KERNEL_DEBUG_DUMP") and s == 0

                def issue_qk_exp(pr, j):
                    pp = j - 4 * s  # >=0: diagonal tile needing mask
                    woff = max(0, pp) * P
                    es_list = []
                    for r01 in range(2):
                        ro = DIM * r01
                        st_ps = ps.tile([P, 512], f32, tag="b512",
                                        name=f"st_ps{r01}")
                        nc.tensor.matmul(
                            st_ps,
                            kt[pr][ro:ro + DIM, j * P:(j + 1) * P],
                            qt[pr][ro:ro + DIM, ls],
                            start=True, stop=True, tile_position=(ro, 0),
                        )
                        es = esp.tile([P, 512], bf16, tag="es")
                        nc.scalar.activation(es[:, woff:], st_ps[:, woff:],
                                             AF.Exp, scale=1.0 / DIM)
                        if pp >= 0:
                            blk = slice(pp * P, (pp + 1) * P)
                            nc.vector.tensor_tensor(
                                es[:, blk], es[:, blk], up01, ALU.mult)
                        if dbg and pr == 0 and j == 0 and r01 == 0:
                            dbg_es = nc.dram_tensor(
                                "dbg_es", [P, 512], bf16,
                                kind="ExternalOutput").ap()
                            nc.sync.dma_start(dbg_es, es)
                        es_list.append(es)
                    return woff, es_list

                def issue_av(pr, j, woff, es_list):
                    if j == 0:
                        av_pair[pr] = ps.tile([DIM + 1, 1024], f32, tag="op",
                                              bufs=1, name="av_pair")
                    for r01 in range(2):
                        hl = 2 * pr + r01
                        nc.tensor.matmul(
                            av_pair[pr][:, r01 * 512 + woff:(r01 + 1) * 512],
                            v_sb[:, j, hl],
                            es_list[r01][:, woff:],
                            start=(j == 0), stop=(j == njs - 1),
                        )

                def emit_tail(pr, av):
                    if dbg and pr == 0:
                        dbg_avn = nc.dram_tensor(
                            "dbg_avn", [DIM, 2, 512], bf16,
                            kind="ExternalOutput").ap()
                        dbg_den = nc.dram_tensor(
                            "dbg_den", [1, 2, 512], f32,
                            kind="ExternalOutput").ap()
                    for r01 in range(2):
                        ro = DIM * r01
                        vsl = slice(r01 * 512, (r01 + 1) * 512)
                        av_sb = esp.tile([DIM, 512], bf16, tag="avsb",
                                         bufs=2)
                        nc.vector.tensor_copy(av_sb, av[:DIM, vsl])
                        # reciprocal_approx_fast (custom DVE uops) corrupts
                        # data for input APs at a nonzero partition base, so
                        # stage the denominator row down to partition 0 first.
                        den_sb = stat.tile([1, 512], f32, tag="den")
                        nc.vector.tensor_copy(den_sb, av[DIM:DIM + 1, vsl])
                        recip = stat.tile([1, 512], f32, tag="recip")
                        nc.vector.reciprocal_approx_fast(recip, den_sb)
                        if dbg and pr == 0:
                            nc.sync.dma_start(dbg_avn[:, r01], av_sb)
                            nc.sync.dma_start(dbg_den[:, r01], den_sb)
                        if dbg and pr == 0 and r01 == 0:
                            dbg_rc = nc.dram_tensor(
                                "dbg_recip", [1, 512], f32,
                                kind="ExternalOutput").ap()
                            nc.sync.dma_start(dbg_rc, recip)
                        bc_ps = ps.tile([DIM, 512], f32, tag="b512",
                                        name="bc_ps")
                        nc.tensor.matmul(bc_ps, ones_row, recip,
                                         start=True, stop=True)
                        nc.vector.tensor_tensor(at[pr][ro:ro + DIM, ls],
                                                av_sb, bc_ps, ALU.mult)

                flat = [(pr, j) for pr in range(2) for j in range(njs)]
                pend = []
                tail_q = []

                def step_tails():
                    for ent in list(tail_q):
                        if ent[2] <= 0:
                            emit_tail(ent[0], ent[1])
                            tail_q.remove(ent)
                        else:
                            ent[2] -= 1

                def pop_av():
                    pr0, j0, woff0, es0 = pend.pop(0)
                    issue_av(pr0, j0, woff0, es0)
                    if j0 == njs - 1:
                        # delay 0: the tail MUST be emitted before the next
                        # "op"-tag allocation reuses this av_pair's PSUM slot
                        # (readers added after slot reuse are untracked).
                        tail_q.append([pr0, av_pair.pop(pr0), 0])

                for pr, j in flat:
                    pend.append((pr, j, *issue_qk_exp(pr, j)))
                    step_tails()
                    if len(pend) > 2:
                        pop_av()
                while pend:
                    pop_av()
                    step_tails()
                for ent in tail_q:
                    emit_tail(ent[0], ent[1])

            def phase_c(s):
                # output projection for supertile s's l-tiles
                for t in range(4 * s, 4 * s + 4):
                    op_ps = ps.tile([P, D], f32, tag="op", bufs=1,
                                    name="op_ps")
                    for nch in range(2):
                        for c in range(2):
                            nc.tensor.matmul(
                                op_ps[:, nch * 512:(nch + 1) * 512],
                                at[c][:, t * P:(t + 1) * P],
                                wout[:, c, nch * 512:(nch + 1) * 512],
                                start=(c == 0), stop=(c == 1),
                            )
                    o_sb = outp.tile([P, D], bf16, tag="o")
                    # 1/32 (v proj) * 1/32 (out proj) = 1/1024
                    nc.vector.tensor_scalar(
                        o_sb, op_ps, 1.0 / 1024.0, None, op0=ALU.mult)
                    nc.sync.dma_start(OUT[t * P:(t + 1) * P, :], o_sb)

            for s in range(SUP):
                qn_tiles = proj_qk_ln(s)
                if s > 0:
                    phase_c(s - 1)
                proj_v(s)
                transposes(s, qn_tiles)
                phase_b(s)
            phase_c(SUP - 1)

            if os.environ.get("KERNEL_DEBUG_DUMP"):
                for nm, tl in (("qt0", qt[0]), ("kt0", kt[0]),
                               ("at0", at[0]), ("at1", at[1])):
                    dbg = nc.dram_tensor(f"dbg_{nm}", [P, L], bf16,
                                         kind="ExternalOutput").ap()
                    nc.sync.dma_start(dbg, tl)
                dbgv = nc.dram_tensor("dbg_v", [P, LT, NHL, DIM + 1], bf16,
                                      kind="ExternalOutput").ap()
                nc.sync.dma_start(dbgv, v_sb)

        outp.release()
        esp.release()
        stat.release()
        work.release()
        big.release()
        const.release()

    nc.finalize()
    return nc


def _get_nc():
    if "nc" not in _CACHE:
        _CACHE["nc"] = _build_nc()
    return _CACHE["nc"]


def kernel(**inputs):
    import ml_dtypes

    bf16 = ml_dtypes.bfloat16
    x = np.asarray(inputs["inputs"], dtype=np.float32)
    w_qk = np.asarray(inputs["W_qk"], dtype=np.float32)
    w_v = np.asarray(inputs["W_v"], dtype=np.float32)
    w_out = np.asarray(inputs["W_out"], dtype=np.float32)

    xt = [np.ascontiguousarray(x[b].T).astype(bf16) for b in range(B)]

    nc = _get_nc()
    in_maps = []
    for c in range(N_CORES):
        b, g = divmod(c, 4)
        in_maps.append({
            "xt": xt[b],
            "w_qk": np.ascontiguousarray(
                w_qk[:, 512 * g:512 * (g + 1)]).astype(bf16),
            "w_v": np.ascontiguousarray(
                w_v[:, 256 * g:256 * (g + 1)]).astype(bf16),
            "w_out": np.ascontiguousarray(
                w_out[256 * g:256 * (g + 1), :]).astype(bf16),
        })

    from concourse.bass_utils import run_bass_kernel_spmd

    trace = bool(os.environ.get("KERNEL_TRACE"))
    if trace:
        try:
            from antenv.axon_hooks import get_axon_ntff_profile_hook
            if get_axon_ntff_profile_hook() is None:
                trace = False
        except Exception:
            trace = False
    res = run_bass_kernel_spmd(nc, in_maps, core_ids=list(range(N_CORES)),
                               trace=trace)
    _CACHE["last_results"] = res
    outs = [m["out"].astype(np.float32) for m in res.results]
    out = np.stack([
        outs[0] + outs[1] + outs[2] + outs[3],
        outs[4] + outs[5] + outs[6] + outs[7],
    ]).astype(np.float32)
    return out


# revision 20
# speedup vs baseline: 2.1455x; 1.2902x over previous
"""Trainium2 Bass kernel for nn_Causal_Attention_13082470383895.

Full (unsharded) inputs in, full output out. Internally shards batch*heads
across 8 NeuronCores: core c owns batch c//4 and the 4 heads [4*(c%4), 4*(c%4)+4).
Each core computes its heads' q/k/v projections (column-sharded weights),
QK-layernorm, causal unnormalized-exp attention, and its partial contribution
to the output projection (row-sharded W_out). Host sums the 4 partials per batch.

All matmul operands are bf16 (PSUM accumulates fp32). x is pre-transposed and
cast to bf16 on the host, so the kernel needs no x transposes. Output partials
are bf16; the host sums them in fp32.

Schedule: the attention phase for supertile s is ACT(exp)-bound while the
projection/layernorm phase is PE-bound, so phase_a(s+1) and phase_c(s-1) are
emitted as PE "filler" interleaved between attention items — the PE works on
projections while the ACT catches up on exp, and AV(j) runs in the shadow of
QK(j+1)+filler. The softmax tail is split: the PSUM reads (av/den copies) are
emitted immediately (before the av PSUM slot is recycled), the
reciprocal/broadcast/normalize two items later so the PE never waits on the
reciprocal. reciprocal_approx_fast needs a partition-0 input AP (the custom
uops corrupt data at a nonzero partition base on HW).

Hardcoded shapes (per spec): inputs [2, 2048, 1024], W_qk [1024, 2048],
W_v [1024, 1024], W_out [1024, 1024], q/k scale=ones, bias=zeros (per spec
fill; layernorm affine is identity and is not applied).
"""

import math
import os
import sys

import numpy as np

sys.path.insert(0, "/opt/trn_rl_repo")

B = 2
L = 2048
D = 1024
HEADS = 16
DIM = 64
LN_EPS = 1e-6
P = 128
LT = L // P          # 16 l-tiles
DT = D // P          # 8 contraction tiles
NHL = 4              # heads per core
SUP = 4              # 512-wide l_q supertiles
N_CORES = 8

_CACHE = {}


def _make_bacc_cls():
    import bass_rust
    import concourse.mybir as mybir
    from concourse import bacc
    from concourse.hw_specs import get_activation_tables

    class KernelBacc(bacc.Bacc):
        """Bacc whose ACT-table selector never picks the `natural_log` set
        for Ln: hiding `ln` there makes the greedy selector choose
        `natural_log_exp_and_others` (which also holds exp/copy/identity/
        square), so the kernel needs a single table load."""

        def insert_act_table_loads(self):
            has_activation = any(
                isinstance(i, mybir.InstActivation)
                for b in self.main_func.blocks
                for i in b.instructions
            )
            if not has_activation:
                return
            ln = mybir.ActivationFunctionType.Ln
            tables = []
            for name, funcs in get_activation_tables(self.m.arch).items():
                if name == "natural_log":
                    funcs = funcs - {ln}
                tables.append((name, funcs))
            bass_rust.insert_act_table_loads(self, tables)

    return KernelBacc


def _build_nc():
    import concourse.bass as bass
    import concourse.mybir as mybir
    import concourse.tile as tile
    from concourse.masks import make_identity, make_upper_triangular

    f32 = mybir.dt.float32
    bf16 = mybir.dt.bfloat16
    AF = mybir.ActivationFunctionType
    ALU = mybir.AluOpType
    AX = mybir.AxisListType

    # layernorm over raw (unscaled) qk groups of 64:
    # (raw - m) / sqrt(var_raw + 1024*eps)  with  m2 = 64*var_raw
    #   rstd = 8 / sqrt(m2 + 64*1024*eps) = exp(-0.5*ln(m2 + EPS2) + ln 8)
    EPS2 = float(DIM * D * LN_EPS)      # 0.065536
    LN8 = float(math.log(8.0))

    nc = _make_bacc_cls()("TRN2", target_bir_lowering=False, debug=False)

    XT = nc.dram_tensor("xt", [D, L], bf16, kind="ExternalInput").ap()
    WQK = nc.dram_tensor("w_qk", [D, 512], bf16, kind="ExternalInput").ap()
    WV = nc.dram_tensor("w_v", [D, 256], bf16, kind="ExternalInput").ap()
    WOUT = nc.dram_tensor("w_out", [256, D], bf16, kind="ExternalInput").ap()
    OUT = nc.dram_tensor("out", [L, D], bf16, kind="ExternalOutput").ap()

    with tile.TileContext(nc) as tc:
        const = tc.alloc_tile_pool(name="const", bufs=1)
        big = tc.alloc_tile_pool(name="big", bufs=1)
        work = tc.alloc_tile_pool(name="work", bufs=2)
        stat = tc.alloc_tile_pool(name="stat", bufs=3)
        esp = tc.alloc_tile_pool(name="esp", bufs=6)
        outp = tc.alloc_tile_pool(name="outp", bufs=2)

        ident = const.tile([P, P], bf16)
        make_identity(nc, ident)
        up01 = const.tile([P, P], bf16)
        make_upper_triangular(nc, up01, val=1.0, diag=True)
        ones_row = const.tile([1, DIM], bf16)
        nc.vector.memset(ones_row, 1.0)
        eps2b = const.tile([P, 1], f32)
        nc.vector.memset(eps2b, EPS2)
        ln8b = const.tile([P, 1], f32)
        nc.vector.memset(ln8b, LN8)

        tc.strict_bb_all_engine_barrier()

        xt = big.tile([P, DT, L], bf16)
        xt_src = XT.rearrange("(c p) l -> p c l", p=P)
        for c in range(DT):
            nc.sync.dma_start(xt[:, c], xt_src[:, c])
        wqk = big.tile([P, DT, 512], bf16)
        nc.sync.dma_start(wqk, WQK.rearrange("(c p) n -> p c n", p=P))
        wv = big.tile([P, DT, 256], bf16)
        nc.sync.dma_start(wv, WV.rearrange("(c p) n -> p c n", p=P))
        wout = big.tile([P, 2, D], bf16)
        nc.sync.dma_start(wout, WOUT.rearrange("(c p) n -> p c n", p=P))

        v_sb = big.tile([P, LT, NHL, DIM + 1], bf16)
        qt = [big.tile([P, L], bf16, name=f"qt{i}") for i in range(2)]
        kt = [big.tile([P, L], bf16, name=f"kt{i}") for i in range(2)]
        at = [big.tile([P, L], bf16, name=f"at{i}") for i in range(2)]
        nc.vector.memset(v_sb[:, :, :, DIM], 1.0)

        # PSUM layout: "st" (attention scores) 3x2KB, "pa" (projection /
        # transpose / broadcast) 3x2KB, "op" (av accumulators + out-proj) 4KB.
        with tc.tile_pool(name="ps", bufs=3, space="PSUM") as ps:

            # ---------- phase_a / phase_c emission units ----------

            def unit_qk_proj(t, chalf):
                """Half of one l-tile's qk projection (4 contraction mms)."""
                tsl = slice(t * P, (t + 1) * P)
                if chalf == 0:
                    qk_ps = ps.tile([P, 512], f32, tag="pa", bufs=3,
                                    name="qk_ps")
                    unit_qk_proj.live[t] = qk_ps
                else:
                    qk_ps = unit_qk_proj.live[t]
                for c in range(4 * chalf, 4 * chalf + 4):
                    nc.tensor.matmul(
                        qk_ps, xt[:, c, tsl], wqk[:, c],
                        start=(c == 0), stop=(c == DT - 1),
                    )

            unit_qk_proj.live = {}

            def unit_ln(t):
                """Layernorm chain for tile t; returns bf16 normalized qn."""
                qk_ps = unit_qk_proj.live.pop(t)
                qk3 = qk_ps.rearrange("p (g d) -> p g d", g=8)
                sum_ = stat.tile([P, 8], f32, tag="sum")
                nc.vector.tensor_reduce(sum_, qk3, AX.X, ALU.add)
                sq = work.tile([P, 8, DIM], f32, tag="sq", bufs=2)
                nc.scalar.activation(sq, qk3, AF.Square)
                ssq = stat.tile([P, 8], f32, tag="ssq")
                nc.vector.tensor_reduce(ssq, sq, AX.X, ALU.add)
                t1 = stat.tile([P, 8], f32, tag="t1")
                nc.vector.tensor_tensor(t1, sum_, sum_, ALU.mult)
                m2 = stat.tile([P, 8], f32, tag="m2")
                nc.vector.scalar_tensor_tensor(
                    m2, t1, -1.0 / DIM, ssq, op0=ALU.mult, op1=ALU.add)
                rstd = stat.tile([P, 8], f32, tag="rstd")
                nc.scalar.activation(rstd, m2, AF.Ln, bias=eps2b, scale=1.0)
                nc.scalar.activation(rstd, rstd, AF.Exp, bias=ln8b,
                                     scale=-0.5)
                negprod = stat.tile([P, 8], f32, tag="negprod")
                nc.vector.scalar_tensor_tensor(
                    negprod, sum_, -1.0 / DIM, rstd,
                    op0=ALU.mult, op1=ALU.mult)
                qn = work.tile([P, 8, DIM], bf16, tag="qn", bufs=6, name="qn")
                # normalize split across ACT (groups 0-3) and DVE (4-7)
                for g in range(4):
                    nc.scalar.activation(
                        qn[:, g], qk3[:, g], AF.Identity,
                        bias=negprod[:, g:g + 1], scale=rstd[:, g:g + 1])
                for g in range(4, 8):
                    nc.vector.tensor_scalar(
                        qn[:, g], qk3[:, g],
                        rstd[:, g:g + 1], negprod[:, g:g + 1],
                        op0=ALU.mult, op1=ALU.add)
                return qn

            def unit_v_proj(t, chalf):
                tsl = slice(t * P, (t + 1) * P)
                if chalf == 0:
                    v_ps = ps.tile([P, 512], f32, tag="pa", bufs=3,
                                   name="v_ps")
                    unit_v_proj.live[t] = v_ps
                else:
                    v_ps = unit_v_proj.live[t]
                for c in range(4 * chalf, 4 * chalf + 4):
                    nc.tensor.matmul(
                        v_ps[:, :256], xt[:, c, tsl], wv[:, c],
                        start=(c == 0), stop=(c == DT - 1),
                    )
                if chalf == 1:
                    nc.vector.tensor_copy(
                        v_sb[:, t, :, :DIM],
                        v_ps[:, :256].rearrange("p (h d) -> p h d", h=NHL))
                    del unit_v_proj.live[t]

            unit_v_proj.live = {}

            def unit_transpose(s, qn_tiles, hl, which):
                pr, ro = hl // 2, DIM * (hl % 2)
                dst = qt if which == 0 else kt
                tp_ps = ps.tile([DIM, 512], bf16, tag="pa", bufs=3,
                                name="tp_ps")
                for i in range(4):
                    nc.tensor.transpose(
                        tp_ps[:, i * P:(i + 1) * P],
                        qn_tiles[i][:, 2 * hl + which],
                        ident,
                    )
                nc.vector.tensor_copy(
                    dst[pr][ro:ro + DIM, s * 512:(s + 1) * 512], tp_ps)

            def unit_out_proj(t):
                op_ps = ps.tile([P, D], f32, tag="op", bufs=1, name="op_ps")
                for nch in range(2):
                    for c in range(2):
                        nc.tensor.matmul(
                            op_ps[:, nch * 512:(nch + 1) * 512],
                            at[c][:, t * P:(t + 1) * P],
                            wout[:, c, nch * 512:(nch + 1) * 512],
                            start=(c == 0), stop=(c == 1),
                        )
                o_sb = outp.tile([P, D], bf16, tag="o")
                # 1/32 (v proj) * 1/32 (out proj) = 1/1024
                nc.scalar.mul(o_sb, op_ps, 1.0 / 1024.0)
                nc.sync.dma_start(OUT[t * P:(t + 1) * P, :], o_sb)

            def gen_phase_a_units(s):
                """Yield emission thunks for supertile s's projections/LN/
                transposes, in dependency-friendly order."""
                qn_tiles = []

                def ln_unit(t):
                    def go():
                        qn_tiles.append(unit_ln(t))
                    return go

                for t in range(4 * s, 4 * s + 4):
                    yield lambda t=t: unit_qk_proj(t, 0)
                    yield lambda t=t: unit_qk_proj(t, 1)
                    yield ln_unit(t)
                for t in range(4 * s, 4 * s + 4):
                    yield lambda t=t: unit_v_proj(t, 0)
                    yield lambda t=t: unit_v_proj(t, 1)
                for hl in range(NHL):
                    for which in range(2):
                        yield (lambda hl=hl, which=which:
                               unit_transpose(s, qn_tiles, hl, which))

            def gen_phase_c_units(s):
                for t in range(4 * s, 4 * s + 4):
                    yield lambda t=t: unit_out_proj(t)

            # ---------- attention ----------

            def phase_b(s, filler):
                """Attention for supertile s; `filler` is a list of thunks
                (next supertile's projections + previous out-projection)
                drained evenly across the attention items as PE filler."""
                ls = slice(s * 512, (s + 1) * 512)
                njs = 4 * s + 4
                av_pair = {}

                def issue_qk_exp(pr, j):
                    pp = j - 4 * s
                    woff = max(0, pp) * P
                    es_list = []
                    for r01 in range(2):
                        ro = DIM * r01
                        st_ps = ps.tile([P, 512], f32, tag="st", bufs=3,
                                        name=f"st_ps{r01}")
                        nc.tensor.matmul(
                            st_ps,
                            kt[pr][ro:ro + DIM, j * P:(j + 1) * P],
                            qt[pr][ro:ro + DIM, ls],
                            start=True, stop=True, tile_position=(ro, 0),
                        )
                        es = esp.tile([P, 512], bf16, tag="es")
                        nc.scalar.activation(es[:, woff:], st_ps[:, woff:],
                                             AF.Exp, scale=1.0 / DIM)
                        if pp >= 0:
                            blk = slice(pp * P, (pp + 1) * P)
                            nc.vector.tensor_tensor(
                                es[:, blk], es[:, blk], up01, ALU.mult)
                        es_list.append(es)
                    return woff, es_list

                def issue_av(pr, j, woff, es_list):
                    if j == 0:
                        av_pair[pr] = ps.tile([DIM + 1, 1024], f32, tag="op",
                                              bufs=1, name="av_pair")
                    for r01 in range(2):
                        hl = 2 * pr + r01
                        nc.tensor.matmul(
                            av_pair[pr][:, r01 * 512 + woff:(r01 + 1) * 512],
                            v_sb[:, j, hl],
                            es_list[r01][:, woff:],
                            start=(j == 0), stop=(j == njs - 1),
                        )

                def tail_copies(pr, av):
                    """PSUM reads — must be emitted before the av slot is
                    recycled by the next op-tag allocation."""
                    out = []
                    for r01 in range(2):
                        vsl = slice(r01 * 512, (r01 + 1) * 512)
                        av_sb = esp.tile([DIM, 512], bf16, tag="avsb",
                                         bufs=4)
                        nc.vector.tensor_copy(av_sb, av[:DIM, vsl])
                        den_sb = stat.tile([1, 512], f32, tag="den")
                        nc.vector.tensor_copy(den_sb, av[DIM:DIM + 1, vsl])
                        out.append((av_sb, den_sb))
                    return out

                def tail_norm(pr, staged):
                    for r01 in range(2):
                        ro = DIM * r01
                        av_sb, den_sb = staged[r01]
                        recip = stat.tile([1, 512], f32, tag="recip")
                        nc.vector.reciprocal_approx_fast(recip, den_sb)
                        recip_bf = stat.tile([1, 512], bf16, tag="recipb")
                        nc.vector.tensor_copy(recip_bf, recip)
                        bc_ps = ps.tile([DIM, 512], f32, tag="pa", bufs=3,
                                        name="bc_ps")
                        nc.tensor.matmul(bc_ps, ones_row, recip_bf,
                                         start=True, stop=True)
                        nc.vector.tensor_tensor(at[pr][ro:ro + DIM, ls],
                                                av_sb, bc_ps, ALU.mult)

                flat = [(pr, j) for pr in range(2) for j in range(njs)]
                pend = []
                norm_q = []
                n_items = len(flat)
                fill_left = list(filler)

                def emit_fill(i):
                    k = -(-len(fill_left) // max(1, n_items - i))  # ceil
                    for _ in range(min(k, len(fill_left))):
                        fill_left.pop(0)()

                def step_norms():
                    for ent in list(norm_q):
                        if ent[2] <= 0:
                            tail_norm(ent[0], ent[1])
                            norm_q.remove(ent)
                        else:
                            ent[2] -= 1

                def pop_av():
                    pr0, j0, woff0, es0 = pend.pop(0)
                    issue_av(pr0, j0, woff0, es0)
                    if j0 == njs - 1:
                        staged = tail_copies(pr0, av_pair.pop(pr0))
                        norm_q.append([pr0, staged, 2])

                for i, (pr, j) in enumerate(flat):
                    pend.append((pr, j, *issue_qk_exp(pr, j)))
                    emit_fill(i)
                    step_norms()
                    if len(pend) > 1:
                        pop_av()
                while pend:
                    pop_av()
                    step_norms()
                for _ in range(3):
                    step_norms()
                for f in fill_left:
                    f()

            # ---------- main schedule ----------

            # supertile 0's projections run standalone (nothing to overlap)
            for f in gen_phase_a_units(0):
                f()
            for s in range(SUP):
                # phase_c units share the "op" PSUM slot with the av
                # accumulators, so they must NOT interleave into the
                # attention items — emit them after the drain instead.
                filler = list(gen_phase_a_units(s + 1)) if s + 1 < SUP else []
                phase_b(s, filler)
                if s > 0:
                    for f in gen_phase_c_units(s - 1):
                        f()
            for f in gen_phase_c_units(SUP - 1):
                f()

        outp.release()
        esp.release()
        stat.release()
        work.release()
        big.release()
        const.release()

    nc.finalize()
    return nc


def _get_nc():
    if "nc" not in _CACHE:
        _CACHE["nc"] = _build_nc()
    return _CACHE["nc"]


def kernel(**inputs):
    import ml_dtypes

    bf16 = ml_dtypes.bfloat16
    x = np.asarray(inputs["inputs"], dtype=np.float32)
    w_qk = np.asarray(inputs["W_qk"], dtype=np.float32)
    w_v = np.asarray(inputs["W_v"], dtype=np.float32)
    w_out = np.asarray(inputs["W_out"], dtype=np.float32)

    xt = [np.ascontiguousarray(x[b].T).astype(bf16) for b in range(B)]

    nc = _get_nc()
    in_maps = []
    for c in range(N_CORES):
        b, g = divmod(c, 4)
        in_maps.append({
            "xt": xt[b],
            "w_qk": np.ascontiguousarray(
                w_qk[:, 512 * g:512 * (g + 1)]).astype(bf16),
            "w_v": np.ascontiguousarray(
                w_v[:, 256 * g:256 * (g + 1)]).astype(bf16),
            "w_out": np.ascontiguousarray(
                w_out[256 * g:256 * (g + 1), :]).astype(bf16),
        })

    from concourse.bass_utils import run_bass_kernel_spmd

    trace = bool(os.environ.get("KERNEL_TRACE"))
    if trace:
        try:
            from antenv.axon_hooks import get_axon_ntff_profile_hook
            if get_axon_ntff_profile_hook() is None:
                trace = False
        except Exception:
            trace = False
    res = run_bass_kernel_spmd(nc, in_maps, core_ids=list(range(N_CORES)),
                               trace=trace)
    _CACHE["last_results"] = res
    outs = [m["out"].astype(np.float32) for m in res.results]
    out = np.stack([
        outs[0] + outs[1] + outs[2] + outs[3],
        outs[4] + outs[5] + outs[6] + outs[7],
    ]).astype(np.float32)
    return out
